# revision 14
# baseline (speedup 1.0000x reference)
"""Distributed kNN novelty-score kernel for Trainium2 (8 NeuronCores).

Problem: emb_state (256, 512), memory (200000, 512), K=5.
  d2[q, n] = ||q||^2 + ||m_n||^2 - 2 q.m_n
  score = mean over (q, k) of sqrt(d2 of the 5 nearest memory rows)

Strategy (memory rows sharded 8 ways, 25000 rows/core):
  - Host per core: tile the memory shard transpose into
    mem_t [P, NG, GD, KT, FD] float16 (contiguous 28KB DMA lines), compute
    nsq2 [2, NSHP] float16 = hi/lo split of -||m||^2 (pad -3e4 in hi, d2 of
    padded rows ~1e4+ keeps them out of every top-k), embT2 = (2*emb).T
    float16, sqq = ||q||^2 float32.
  - Device: s[q, n] = 2 q.m - ||m||^2 in PSUM. fp16 streams 1 col/cycle on
    the PE (fp32 is half rate). The -||m||^2 term is a K=2 matmul
    (ones[2,128] x nsq2 chunk) opening each accumulation group. Smallest-d2
    == largest-s, selected exactly with DVE max8 (top-8 per partition,
    sorted desc) straight from PSUM per 512 chunk, then max8 over chunk
    winners -> local top-5 values per query.
  - AllGather the 8x(256x5) candidates, max8 over the 40 per query ->
    global top-5; dist = sqrt(sqq - s) on ACT; mean via ones-matmul
    partition reduction. All cores compute the same scalar.
"""

import sys

sys.path.insert(0, "/opt/trn_rl_repo")

import numpy as np

Q = 256
D = 512
N = 200000
K = 5
NCORES = 8
NSH = N // NCORES        # 25000 memory rows per core
P = 128
KT = D // P              # 4 contraction tiles
QT = Q // P              # 2 query tiles
FD = 512                 # free-dim chunk (one fp32 PSUM bank)
GD = 7                   # chunks per DMA group
NG = 7                   # DMA groups
NCH = GD * NG            # 49
NSHP = NCH * FD          # 25088 (padded shard length)
PAD_NSQ = -30000.0       # fp16-safe pad: s = -30000 never reaches a top-k
STREAM_BUFS = 3

_CACHE = {}


def _build_bass():
    import concourse.bacc as bacc
    import concourse.mybir as mybir
    import concourse.tile as tile

    f32 = mybir.dt.float32
    f16 = mybir.dt.float16
    X = mybir.AxisListType.X

    nc = bacc.Bacc(num_devices=NCORES)
    embT2 = nc.declare_dram_parameter("embT2", [D, Q], f16, isOutput=False)
    mem_t = nc.declare_dram_parameter(
        "mem_t", [P, NG, GD, KT, FD], f16, isOutput=False
    )
    nsq2 = nc.declare_dram_parameter("nsq2", [2, NSHP], f16, isOutput=False)
    sqq = nc.declare_dram_parameter("sqq", [Q, 1], f32, isOutput=False)
    out = nc.declare_dram_parameter("out", [1, 1], f32, isOutput=True)

    with tile.TileContext(nc) as tc:
        with (
            tc.tile_pool(name="const", bufs=1) as cpool,
            tc.tile_pool(name="stream", bufs=STREAM_BUFS) as spool,
            tc.tile_pool(name="top", bufs=1) as tpool,
            tc.tile_pool(name="small", bufs=2) as mpool,
            tc.tile_pool(name="acc", bufs=4, space="PSUM") as ppool,
            tc.tile_pool(name="fin", bufs=1, space="PSUM") as fpool,
            tc.tile_pool(name="dram", bufs=1, space="DRAM") as dpool,
        ):
            # ---- constants ----
            w = cpool.tile([P, KT, Q], f16)
            nc.sync.dma_start(
                out=w[:], in_=embT2[:, :].rearrange("(kt p) q -> p kt q", p=P)
            )
            nsq_sb = cpool.tile([2, NSHP], f16)
            nc.sync.dma_start(out=nsq_sb[:], in_=nsq2[:, :])
            sqq_sb = cpool.tile([P, QT], f32)
            nc.sync.dma_start(
                out=sqq_sb[:],
                in_=sqq[:, :].rearrange("(qt p) one -> p (qt one)", p=P),
            )
            ones2 = cpool.tile([2, P], f16)
            nc.vector.memset(ones2[:], 1.0)
            ones128 = cpool.tile([P, 1], f32)
            nc.vector.memset(ones128[:], 1.0)

            # per-chunk top-8 candidates for every (query, q-tile)
            cand8 = tpool.tile([P, QT, NCH, 8], f32)

            for g in range(NG):
                mt = spool.tile([P, GD, KT, FD], f16, tag="memtile")
                nc.sync.dma_start(out=mt[:], in_=mem_t[:, g, :, :, :])
                for c in range(GD):
                    ch = g * GD + c
                    for qt in range(QT):
                        ps = ppool.tile([P, FD], f32, tag="acc")
                        # psum = -||m||^2 (hi+lo, K=2), then += 2 q.m
                        nc.tensor.matmul(
                            ps[:],
                            ones2[:],
                            nsq_sb[:, ch * FD : (ch + 1) * FD],
                            start=True,
                            stop=False,
                        )
                        for kt in range(KT):
                            nc.tensor.matmul(
                                ps[:],
                                w[:, kt, qt * P : (qt + 1) * P],
                                mt[:, c, kt, :],
                                start=False,
                                stop=(kt == KT - 1),
                            )
                        nc.vector.max(cand8[:, qt, ch, :], ps[:])

            # ---- local top-5 -> internal DRAM (single DMA) ----
            loc = dpool.tile([QT, P, K], f32)
            l8all = mpool.tile([P, QT, 8], f32, tag="l8")
            for qt in range(QT):
                nc.vector.max(l8all[:, qt, :], cand8[:, qt, :, :])
            nc.sync.dma_start(
                out=loc[:].rearrange("qt p k -> p qt k"), in_=l8all[:, :, 0:K]
            )

            # ---- exchange candidates ----
            allc = dpool.tile([NCORES, QT, P, K], f32, addr_space="Shared")
            nc.gpsimd.collective_compute(
                "AllGather",
                mybir.AluOpType.bypass,
                replica_groups=[list(range(NCORES))],
                ins=[loc[:].opt()],
                outs=[allc[:].opt()],
            )

            # ---- global top-5 and score ----
            red = tpool.tile([P, QT], f32)
            for qt in range(QT):
                gg = mpool.tile([P, NCORES, K], f32, tag="gg")
                nc.sync.dma_start(
                    out=gg[:], in_=allc[:, qt, :, :].rearrange("c p k -> p c k")
                )
                g8 = mpool.tile([P, 8], f32, tag="g8")
                nc.vector.max(g8[:], gg[:])
                dist = mpool.tile([P, K], f32, tag="dist")
                # dist = sqrt(-s + ||q||^2) = sqrt(d2)
                nc.scalar.activation(
                    dist[:],
                    g8[:, 0:K],
                    mybir.ActivationFunctionType.Sqrt,
                    bias=sqq_sb[:, qt : qt + 1],
                    scale=-1.0,
                )
                nc.vector.reduce_sum(red[:, qt : qt + 1], dist[:], axis=X)

            pfin = fpool.tile([1, QT], f32)
            nc.tensor.matmul(pfin[:], ones128[:], red[:], start=True, stop=True)
            fin = mpool.tile([1, 1], f32, tag="fin")
            nc.vector.reduce_sum(fin[:], pfin[:], axis=X)
            nc.scalar.mul(fin[:], fin[:], 1.0 / (Q * K))
            nc.sync.dma_start(out=out[:, :], in_=fin[:])

    # Legalize sync waits (walrus enforces <=1 per instruction):
    # move_matmul_waits_to_ldweights + generate_event_semaphores.
    nc.compile()
    return nc


def _get_bass():
    if "nc" not in _CACHE:
        _CACHE["nc"] = _build_bass()
    return _CACHE["nc"]


def make_in_maps(emb_state: np.ndarray, memory: np.ndarray):
    """Shard + lay out inputs for the 8 cores."""
    emb_state = np.asarray(emb_state, dtype=np.float32)
    memory = np.asarray(memory, dtype=np.float32)
    embT2 = np.ascontiguousarray((2.0 * emb_state).T).astype(np.float16)
    sqq = np.sum(emb_state * emb_state, axis=1).reshape(Q, 1)

    in_maps = []
    for c in range(NCORES):
        m = memory[c * NSH : (c + 1) * NSH]                    # [25000, 512]
        mp = np.zeros((NSHP, D), dtype=np.float32)
        mp[:NSH] = m
        # mem_t[p, g, gd, kt, f] = mp[(g*GD+gd)*FD+f, kt*P+p]
        mt = np.ascontiguousarray(
            mp.reshape(NG, GD, FD, KT, P).transpose(4, 0, 1, 3, 2)
        ).astype(np.float16)
        nsq = np.full(NSHP, PAD_NSQ, dtype=np.float32)
        nsq[:NSH] = -np.sum(m.astype(np.float64) * m, axis=1).astype(np.float32)
        hi = nsq.astype(np.float16)
        lo = (nsq - hi.astype(np.float32)).astype(np.float16)
        nsq2 = np.stack([hi, lo], axis=0)                      # [2, NSHP] f16
        in_maps.append(
            {"embT2": embT2, "mem_t": mt, "nsq2": nsq2, "sqq": sqq.copy()}
        )
    return in_maps


def _install_ntff_hook():
    """Register the axon NTFF profile hook that this container's antenv lacks."""
    import sys as _sys
    import types

    if "antenv.axon_hooks" in _sys.modules:
        return
    try:
        import antenv
        from trn_agent_boot.trn_boot import _ntff_profile_via_ctypes

        hook = _ntff_profile_via_ctypes("/opt/axon/libaxon_pjrt.so")
        mod = types.ModuleType("antenv.axon_hooks")
        mod.get_axon_ntff_profile_hook = lambda: hook
        mod.set_axon_ntff_profile_hook = lambda h: None
        _sys.modules["antenv.axon_hooks"] = mod
        antenv.axon_hooks = mod
    except Exception as e:  # profiling is best-effort
        print(f"ntff hook install failed: {e}")


def _run(in_maps, trace=False):
    from concourse.bass_utils import run_bass_kernel_spmd

    if trace:
        _install_ntff_hook()
    nc = _get_bass()
    res = run_bass_kernel_spmd(
        nc, in_maps, core_ids=list(range(NCORES)), trace=trace
    )
    return res


def kernel(emb_state: np.ndarray, memory: np.ndarray) -> np.ndarray:
    in_maps = make_in_maps(emb_state, memory)
    res = _run(in_maps, trace=False)
    val = np.float32(res.results[0]["out"].reshape(-1)[0])
    return np.asarray(val, dtype=np.float32).reshape(())


# revision 16
# speedup vs baseline: 1.1816x; 1.1816x over previous
"""Distributed kNN novelty-score kernel for Trainium2 (8 NeuronCores).

Problem: emb_state (256, 512), memory (200000, 512), K=5.
  d2[q, n] = ||q||^2 + ||m_n||^2 - 2 q.m_n
  score = mean over (q, k) of sqrt(d2 of the 5 nearest memory rows)

Strategy (memory rows sharded 8 ways, 25000 rows/core):
  - Host per core: tile the memory shard transpose into
    mem_t [P, NCH, KT, FD] float16 (8KB contiguous DMA lines per chunk),
    nsq2 [2, NSHP] float16 = hi/lo split of -||m||^2 (pad -3e4: padded rows
    get s = -3e4 and never reach a top-k), embT2 = (2*emb).T float16,
    sqq = ||q||^2 float32.
  - Device: s[q, n] = 2 q.m - ||m||^2 in PSUM (fp16 streams 1 col/cycle on
    the PE, 1024-col moving operand = two fp32 PSUM banks per group). The
    -||m||^2 term is a K=2 matmul (ones[2,128] x nsq2 chunk) opening each
    accumulation group. Smallest-d2 == largest-s, selected exactly with DVE
    max8 (top-8 per partition, sorted desc) straight from PSUM per chunk,
    then max8 over chunk winners -> local top-5 values per query.
  - AllGather the 8x(256x5) candidates, max8 over the 40 per query ->
    global top-5; dist = sqrt(sqq - s) on ACT; mean via ones-matmul
    partition reduction. All cores compute the same scalar.
"""

import sys

sys.path.insert(0, "/opt/trn_rl_repo")

import numpy as np

Q = 256
D = 512
N = 200000
K = 5
NCORES = 8
NSH = N // NCORES        # 25000 memory rows per core
P = 128
KT = D // P              # 4 contraction tiles
QT = Q // P              # 2 query tiles
FD = 512                 # free-dim chunk (one fp32 PSUM bank)
NCH = 49                 # chunks
NSHP = NCH * FD          # 25088 (padded shard length)
KTE = KT + 1             # 4 data k-tiles + 1 nsq k-tile
G_SIZES = (2, 4, 8, 8, 9, 9, 9)  # chunks per DMA group (small first = fast fill)
PAD_NSQ = -30000.0       # fp16-safe pad
STREAM_BUFS = 3

assert sum(G_SIZES) == NCH

_CACHE = {}


def _build_bass():
    import concourse.bacc as bacc
    import concourse.mybir as mybir
    import concourse.tile as tile

    f32 = mybir.dt.float32
    f16 = mybir.dt.float16
    X = mybir.AxisListType.X

    nc = bacc.Bacc(num_devices=NCORES)
    embT2 = nc.declare_dram_parameter("embT2", [D, Q], f16, isOutput=False)
    mem_t = nc.declare_dram_parameter(
        "mem_t", [P, NCH, KTE, FD], f16, isOutput=False
    )
    onesw = nc.declare_dram_parameter("onesw", [P, P], f16, isOutput=False)
    sqq = nc.declare_dram_parameter("sqq", [Q, 1], f32, isOutput=False)
    out = nc.declare_dram_parameter("out", [1, 1], f32, isOutput=True)

    with tile.TileContext(nc) as tc:
        with (
            tc.tile_pool(name="const", bufs=1) as cpool,
            tc.tile_pool(name="stream", bufs=STREAM_BUFS) as spool,
            tc.tile_pool(name="top", bufs=1) as tpool,
            tc.tile_pool(name="small", bufs=2) as mpool,
            tc.tile_pool(name="acc", bufs=6, space="PSUM") as ppool,
            tc.tile_pool(name="fin", bufs=1, space="PSUM") as fpool,
            tc.tile_pool(name="dram", bufs=1, space="DRAM") as dpool,
        ):
            # ---- constants ----
            w = cpool.tile([P, KT, Q], f16)
            nc.sync.dma_start(
                out=w[:], in_=embT2[:, :].rearrange("(kt p) q -> p kt q", p=P)
            )
            onesw_sb = cpool.tile([P, P], f16)
            nc.sync.dma_start(out=onesw_sb[:], in_=onesw[:, :])
            sqq_sb = cpool.tile([P, QT], f32)
            nc.sync.dma_start(
                out=sqq_sb[:],
                in_=sqq[:, :].rearrange("(qt p) one -> p (qt one)", p=P),
            )
            ones128 = cpool.tile([P, 1], f32)
            nc.vector.memset(ones128[:], 1.0)

            # per-chunk top-8 candidates for every (query, q-tile)
            cand8 = tpool.tile([P, QT, NCH, 8], f32)

            ch0 = 0
            for gsz in G_SIZES:
                mt = spool.tile([P, max(G_SIZES), KTE, FD], f16, tag="memtile")
                nc.sync.dma_start(
                    out=mt[:, 0:gsz, :, :], in_=mem_t[:, ch0 : ch0 + gsz, :, :]
                )
                for c in range(gsz):
                    ch = ch0 + c
                    for qt in range(QT):
                        ps = ppool.tile([P, FD], f32, tag="acc")
                        # psum = -||m||^2: K=128 matmul against the nsq
                        # k-tile (rows 0/1 = hi/lo, rest zero) so FWL stays
                        # enabled, then += 2 q.m over the 4 data k-tiles
                        nc.tensor.matmul(
                            ps[:],
                            onesw_sb[:],
                            mt[:, c, KT, :],
                            start=True,
                            stop=False,
                        )
                        for kt in range(KT):
                            nc.tensor.matmul(
                                ps[:],
                                w[:, kt, qt * P : (qt + 1) * P],
                                mt[:, c, kt, :],
                                start=False,
                                stop=(kt == KT - 1),
                            )
                        nc.vector.max(cand8[:, qt, ch, :], ps[:])
                ch0 += gsz

            # ---- local top-5 -> internal DRAM (single DMA) ----
            loc = dpool.tile([QT, P, K], f32)
            l8all = mpool.tile([P, QT, 8], f32, tag="l8")
            for qt in range(QT):
                nc.vector.max(l8all[:, qt, :], cand8[:, qt, :, :])
            nc.sync.dma_start(
                out=loc[:].rearrange("qt p k -> p qt k"), in_=l8all[:, :, 0:K]
            )

            # ---- exchange candidates ----
            allc = dpool.tile([NCORES, QT, P, K], f32, addr_space="Shared")
            nc.gpsimd.collective_compute(
                "AllGather",
                mybir.AluOpType.bypass,
                replica_groups=[list(range(NCORES))],
                ins=[loc[:].opt()],
                outs=[allc[:].opt()],
            )

            # ---- global top-5 and score ----
            red = tpool.tile([P, QT], f32)
            for qt in range(QT):
                gg = mpool.tile([P, NCORES, K], f32, tag="gg")
                nc.sync.dma_start(
                    out=gg[:], in_=allc[:, qt, :, :].rearrange("c p k -> p c k")
                )
                g8 = mpool.tile([P, 8], f32, tag="g8")
                nc.vector.max(g8[:], gg[:])
                dist = mpool.tile([P, K], f32, tag="dist")
                # dist = sqrt(-s + ||q||^2) = sqrt(d2)
                nc.scalar.activation(
                    dist[:],
                    g8[:, 0:K],
                    mybir.ActivationFunctionType.Sqrt,
                    bias=sqq_sb[:, qt : qt + 1],
                    scale=-1.0,
                )
                nc.vector.reduce_sum(red[:, qt : qt + 1], dist[:], axis=X)

            pfin = fpool.tile([1, QT], f32)
            nc.tensor.matmul(pfin[:], ones128[:], red[:], start=True, stop=True)
            fin = mpool.tile([1, 1], f32, tag="fin")
            nc.vector.reduce_sum(fin[:], pfin[:], axis=X)
            nc.scalar.mul(fin[:], fin[:], 1.0 / (Q * K))
            nc.sync.dma_start(out=out[:, :], in_=fin[:])

    # Legalize sync waits (walrus enforces <=1 per instruction):
    # move_matmul_waits_to_ldweights + generate_event_semaphores.
    nc.compile()
    return nc


def _get_bass():
    if "nc" not in _CACHE:
        _CACHE["nc"] = _build_bass()
    return _CACHE["nc"]


def make_in_maps(emb_state: np.ndarray, memory: np.ndarray):
    """Shard + lay out inputs for the 8 cores."""
    emb_state = np.asarray(emb_state, dtype=np.float32)
    memory = np.asarray(memory, dtype=np.float32)
    embT2 = np.ascontiguousarray((2.0 * emb_state).T).astype(np.float16)
    sqq = np.sum(emb_state * emb_state, axis=1).reshape(Q, 1)

    in_maps = []
    for c in range(NCORES):
        m = memory[c * NSH : (c + 1) * NSH]                    # [25000, 512]
        mp = np.zeros((NSHP, D), dtype=np.float32)
        mp[:NSH] = m
        # mem_t[p, ch, kt, f] = mp[ch*FD+f, kt*P+p] for kt<4;
        # kt=4 rows 0/1 carry the -||m||^2 hi/lo split, rest zeros.
        base = mp.reshape(NCH, FD, KT, P).transpose(3, 0, 2, 1)
        mt = np.zeros((P, NCH, KTE, FD), dtype=np.float16)
        mt[:, :, :KT, :] = base.astype(np.float16)
        nsq = np.full(NSHP, PAD_NSQ, dtype=np.float32)
        nsq[:NSH] = -np.sum(m.astype(np.float64) * m, axis=1).astype(np.float32)
        hi = nsq.astype(np.float16)
        lo = (nsq - hi.astype(np.float32)).astype(np.float16)
        mt[0, :, KT, :] = hi.reshape(NCH, FD)
        mt[1, :, KT, :] = lo.reshape(NCH, FD)
        onesw = np.zeros((P, P), dtype=np.float16)
        onesw[0:2, :] = 1.0
        in_maps.append(
            {"embT2": embT2, "mem_t": mt, "onesw": onesw, "sqq": sqq.copy()}
        )
    return in_maps


def _install_ntff_hook():
    """Register the axon NTFF profile hook that this container's antenv lacks."""
    import sys as _sys
    import types

    if "antenv.axon_hooks" in _sys.modules:
        return
    try:
        import antenv
        from trn_agent_boot.trn_boot import _ntff_profile_via_ctypes

        hook = _ntff_profile_via_ctypes("/opt/axon/libaxon_pjrt.so")
        mod = types.ModuleType("antenv.axon_hooks")
        mod.get_axon_ntff_profile_hook = lambda: hook
        mod.set_axon_ntff_profile_hook = lambda h: None
        _sys.modules["antenv.axon_hooks"] = mod
        antenv.axon_hooks = mod
    except Exception as e:  # profiling is best-effort
        print(f"ntff hook install failed: {e}")


def _run(in_maps, trace=False):
    from concourse.bass_utils import run_bass_kernel_spmd

    if trace:
        _install_ntff_hook()
    nc = _get_bass()
    res = run_bass_kernel_spmd(
        nc, in_maps, core_ids=list(range(NCORES)), trace=trace
    )
    return res


def kernel(emb_state: np.ndarray, memory: np.ndarray) -> np.ndarray:
    in_maps = make_in_maps(emb_state, memory)
    res = _run(in_maps, trace=False)
    val = np.float32(res.results[0]["out"].reshape(-1)[0])
    return np.asarray(val, dtype=np.float32).reshape(())
